# revision 10
# baseline (speedup 1.0000x reference)
"""Multi-head low-rank-score (LSR) causal attention on 8 trn2 NeuronCores.

Sharding: core = 4*b + g owns batch b and heads [4g, 4g+4).
Each core: q/k/v projections for its 256 head-dims, low-rank score
projections, causal softmax attention, and a partial o_proj
(its 256 ctx dims x full 1024 out dims). Host sums the 4 partials
per batch and adds biases.

All matmuls run in float32r (full-rate fp32, ~13-bit mantissa).
Softmax: two score passes -
  pass 1 (q-layout) computes the per-query causal row max on DVE;
  pass 2 (k-layout) computes S^T with the max subtraction and the
  block-level causal mask folded into the matmul via augmented
  contraction rows, then exp() on ScalarE writes P^T straight to SBUF.
AV multiplies V (augmented with a ones column -> softmax denominators
come out of the same matmul) by P^T, and the division is applied via a
reciprocal + rank-2 broadcast matmul + one DVE multiply.
"""

import numpy as np
import ml_dtypes

B = 2
T = 2048
D = 1024
H = 16
DH = 64
R = 32
HPC = 4  # heads per core
OC = HPC * DH  # 256 out-cols per core
NCORES = 8
SCALE = 1.0 / float(np.sqrt(np.float32(R)))
NEG = -30000.0
NT = T // 128  # 16 key/query tiles
NCH = T // 512  # 4 query chunks

_cache = {}


def _build():
    import concourse.bacc as bacc
    import concourse.mybir as mybir
    from concourse.tile import TileContext

    F32 = mybir.dt.float32
    F32R = mybir.dt.float32r
    BF16 = mybir.dt.bfloat16
    EXP = mybir.ActivationFunctionType.Exp
    COPY = mybir.ActivationFunctionType.Copy
    IDENT = mybir.ActivationFunctionType.Identity
    MAX = mybir.AluOpType.max
    AXX = mybir.AxisListType.X

    nc = bacc.Bacc("TRN2", target_bir_lowering=False, debug=False,
                   num_devices=NCORES)

    xT = nc.declare_dram_parameter("xT", [D, T], F32R, isOutput=False)
    wq = nc.declare_dram_parameter("wq", [D, OC], F32R, isOutput=False)
    wk = nc.declare_dram_parameter("wk", [D, OC], F32R, isOutput=False)
    wv = nc.declare_dram_parameter("wv", [D, OC], F32R, isOutput=False)
    wo = nc.declare_dram_parameter("wo", [OC, D], F32R, isOutput=False)
    wql = nc.declare_dram_parameter("wql", [DH, HPC * R], F32R, isOutput=False)
    wkl = nc.declare_dram_parameter("wkl", [DH, HPC * R], F32R, isOutput=False)
    bq = nc.declare_dram_parameter("bq", [OC, 1], F32, isOutput=False)
    bk = nc.declare_dram_parameter("bk", [OC, 1], F32, isOutput=False)
    # [16, T] row j': NEG where t < 128*j' else 0
    indq = nc.declare_dram_parameter("indq", [NT, T], F32R, isOutput=False)
    # [17, T]: row 0 = ones; rows 1+j': 1.0 on k-tile j' cols else 0
    okq = nc.declare_dram_parameter("okq", [NT + 1, T], F32R, isOutput=False)
    triq = nc.declare_dram_parameter("triq", [128, 128], F32, isOutput=False)
    trik = nc.declare_dram_parameter("trik", [128, 128], F32, isOutput=False)
    sel2 = nc.declare_dram_parameter("sel2", [2, 128], F32R, isOutput=False)
    ibf = nc.declare_dram_parameter("ibf", [128, 128], BF16, isOutput=False)
    yT = nc.declare_dram_parameter("yT", [D, T], F32, isOutput=True)

    with TileContext(nc) as tc:
        with (
            nc.allow_low_precision(reason="f32r reciprocal / bf16 row-max"),
            tc.tile_pool(name="persist", bufs=1) as pp,
        ):
            # ---- persistent SBUF tiles
            wq_t = [pp.tile([128, OC], F32R, tag=f"wq{i}", name=f"wq{i}") for i in range(8)]
            wk_t = [pp.tile([128, OC], F32R, tag=f"wk{i}", name=f"wk{i}") for i in range(8)]
            wv_t = [pp.tile([128, OC], F32R, tag=f"wv{i}", name=f"wv{i}") for i in range(8)]
            # lsr weights duplicated at partition bases 0 and 64 (row packing)
            wql_t = pp.tile([128, HPC * R], F32R, tag="wql")
            wkl_t = pp.tile([128, HPC * R], F32R, tag="wkl")
            bq_t = [pp.tile([128, 1], F32, tag=f"bq{i}", name=f"bq{i}") for i in range(2)]
            bk_t = [pp.tile([128, 1], F32, tag=f"bk{i}", name=f"bk{i}") for i in range(2)]
            triq_t = pp.tile([128, 128], F32, tag="triq")
            trik_t = pp.tile([128, 128], F32, tag="trik")
            sel2_t = pp.tile([2, 128], F32R, tag="sel2")
            ibf_t = pp.tile([128, 128], BF16, tag="ibf")
            # augmented lr tiles, one per head pair p (heads 2p, 2p+1)
            # rows [64l, 64l+32): scale*q_lrT / k_lrT of head 2p+l
            # row 64l+32: -m (q side) / ones (k side)
            # rows [64l+33, 64l+49): indq (q side) / selk (k side)
            qaug = [pp.tile([128, T], F32R, tag=f"qaug{p}", name=f"qaug{p}") for p in range(2)]
            kaug = [pp.tile([128, T], F32R, tag=f"kaug{p}", name=f"kaug{p}") for p in range(2)]
            # V augmented with ones column, per head x key tile
            vaug = [[pp.tile([128, DH + 1], F32R, tag=f"va{h}_{j}", name=f"va{h}_{j}")
                     for j in range(NT)] for h in range(HPC)]
            # scaled ctx^T ready for o_proj: [pair][chunk]
            ctxr = [[pp.tile([128, 512], F32R, tag=f"cx{p}_{c}", name=f"cx{p}_{c}")
                     for c in range(NCH)] for p in range(2)]
            wo_t = [pp.tile([128, D], F32R, tag=f"wo{p}", name=f"wo{p}") for p in range(2)]

            for i in range(8):
                nc.sync.dma_start(out=wq_t[i][:], in_=wq[128 * i:128 * i + 128, :])
                nc.sync.dma_start(out=wk_t[i][:], in_=wk[128 * i:128 * i + 128, :])
                nc.sync.dma_start(out=wv_t[i][:], in_=wv[128 * i:128 * i + 128, :])
            for l in range(2):
                nc.sync.dma_start(out=wql_t[64 * l:64 * l + DH, :], in_=wql[:])
                nc.sync.dma_start(out=wkl_t[64 * l:64 * l + DH, :], in_=wkl[:])
            for i in range(2):
                nc.sync.dma_start(out=bq_t[i][:], in_=bq[128 * i:128 * i + 128, :])
                nc.sync.dma_start(out=bk_t[i][:], in_=bk[128 * i:128 * i + 128, :])
            nc.sync.dma_start(out=triq_t[:], in_=triq[:])
            nc.sync.dma_start(out=trik_t[:], in_=trik[:])
            nc.sync.dma_start(out=sel2_t[:], in_=sel2[:])
            nc.sync.dma_start(out=ibf_t[:], in_=ibf[:])
            for p in range(2):
                nc.sync.dma_start(out=wo_t[p][:], in_=wo[128 * p:128 * p + 128, :])
                for l in range(2):
                    nc.sync.dma_start(out=qaug[p][64 * l + 33:64 * l + 49, :],
                                      in_=indq[:])
                    nc.sync.dma_start(out=kaug[p][64 * l + 32:64 * l + 49, :],
                                      in_=okq[:])

            # ---- phase 1: projections (uses xT; QT/KT transient)
            with (
                tc.tile_pool(name="px", bufs=1) as px,
                tc.tile_pool(name="pqk", bufs=2) as pqk,
                tc.tile_pool(name="ps1", bufs=2, space="PSUM") as ps1,
                tc.tile_pool(name="psl", bufs=2, space="PSUM") as psl,
            ):
                xt_t = [px.tile([128, T], F32R, tag=f"x{i}", name=f"x{i}") for i in range(8)]
                for i in range(8):
                    nc.sync.dma_start(out=xt_t[i][:],
                                      in_=xT[128 * i:128 * i + 128, :])

                # V: [t-tile, 256] accumulating 8 k-tiles
                for tt in range(NT):
                    vps = ps1.tile([128, OC], F32, tag="vps")
                    for kk in range(8):
                        nc.tensor.matmul(
                            vps[:], xt_t[kk][:, 128 * tt:128 * tt + 128],
                            wv_t[kk][:], start=(kk == 0), stop=(kk == 7))
                    for h in range(HPC):
                        nc.scalar.copy(vaug[h][tt][:, 0:DH],
                                       vps[:, DH * h:DH * h + DH])
                        nc.vector.memset(vaug[h][tt][:, DH:DH + 1].bitcast(F32), 1.0)

                # QT/KT oc-tiles -> lsr -> aug tiles; QT/KT slots recycled
                for side in range(2):  # 0 = q, 1 = k
                    w_t = wq_t if side == 0 else wk_t
                    b_t = bq_t if side == 0 else bk_t
                    lsr_w = wql_t if side == 0 else wkl_t
                    aug = qaug if side == 0 else kaug
                    evac_scale = SCALE if side == 0 else 1.0
                    for ot in range(2):  # oc tile = head pair p = ot
                        qk_sb = pqk.tile([128, T], F32R, tag="qkt")
                        for ch in range(NCH):
                            pps = ps1.tile([128, 512], F32, tag="pps")
                            for kk in range(8):
                                nc.tensor.matmul(
                                    pps[:],
                                    w_t[kk][:, 128 * ot:128 * ot + 128],
                                    xt_t[kk][:, 512 * ch:512 * ch + 512],
                                    start=(kk == 0), stop=(kk == 7))
                            nc.scalar.activation(
                                qk_sb[:, 512 * ch:512 * ch + 512], pps[:],
                                IDENT, bias=b_t[ot][:], scale=1.0)
                        # lsr for the two heads in this oc tile (row-packed
                        # at partition bases 0 / 64)
                        for ch in range(NCH):
                            for l in range(2):
                                h = 2 * ot + l
                                lps = psl.tile([R, 512], F32, tag=f"lps{l}", name=f"lps{l}")
                                nc.tensor.matmul(
                                    lps[:],
                                    lsr_w[64 * l:64 * l + DH,
                                          R * h:R * h + R],
                                    qk_sb[64 * l:64 * l + DH,
                                          512 * ch:512 * ch + 512],
                                    start=True, stop=True,
                                    tile_position=(64 * l, 0))
                                nc.scalar.activation(
                                    aug[ot][64 * l:64 * l + R,
                                            512 * ch:512 * ch + 512],
                                    lps[:], COPY, scale=evac_scale)

            # ---- phases 2-4 merged: stats / S^T+AV / o_proj interleaved
            # per 512-query chunk so the PE never idles long enough to
            # re-throttle. PSUM: sT0(2) + sT1(1) + st0/st1(2) + av0/av1(2)
            # + nmt(1) = 8 banks; scl shares st0, yps shares st1.
            with (
                tc.tile_pool(name="psw", bufs=1, space="PSUM") as psw,
                tc.tile_pool(name="psT", bufs=2, space="PSUM") as psT,
                tc.tile_pool(name="psav", bufs=1, space="PSUM") as psav,
                tc.tile_pool(name="psn", bufs=1, space="PSUM") as psn,
                tc.tile_pool(name="pmx", bufs=2) as pmx,
                tc.tile_pool(name="pst", bufs=4) as pst,
                tc.tile_pool(name="pcx", bufs=2) as pcx,
            ):
                def emit_stats(c):
                    for ii in range(4):
                        i = 4 * c + ii
                        nchunks = i // 4 + 1
                        mx = pmx.tile([128, HPC * 4], F32, tag="mx",
                                      name="mx")
                        negm = pmx.tile([128, HPC], BF16, tag="negm",
                                        name="negm")
                        for p in range(2):
                            for l in range(2):
                                h = 2 * p + l
                                for cc in range(nchunks):
                                    ncols = min(512, 128 * (i + 1) - 512 * cc)
                                    sps = psw.tile([128, 512], F32,
                                                   tag=f"st{l}",
                                                   name=f"st{l}")
                                    nc.tensor.matmul(
                                        sps[:, 0:ncols],
                                        qaug[p][64 * l:64 * l + R,
                                                128 * i:128 * i + 128],
                                        kaug[p][64 * l:64 * l + R,
                                                512 * cc:512 * cc + ncols],
                                        start=True, stop=True,
                                        tile_position=(64 * l, 0))
                                    if cc == nchunks - 1:
                                        a = ncols - 128
                                        nc.vector.tensor_add(
                                            sps[:, a:a + 128],
                                            sps[:, a:a + 128], triq_t[:])
                                    nc.vector.tensor_reduce(
                                        mx[:, 4 * h + cc:4 * h + cc + 1],
                                        sps[:, 0:ncols], axis=AXX, op=MAX)
                        for h in range(HPC):
                            nc.vector.tensor_reduce(
                                negm[:, h:h + 1],
                                mx[:, 4 * h:4 * h + nchunks],
                                axis=AXX, op=MAX, negate=True)
                        nmt = psn.tile([HPC, 128], BF16, tag="nmt",
                                       name="nmt")
                        nc.tensor.transpose(nmt[:], negm[:], ibf_t[:])
                        nmt_sb = pmx.tile([HPC, 128], F32R, tag="nmt_sb",
                                          name="nmt_sb")
                        nc.scalar.copy(nmt_sb[:], nmt[:])
                        for p in range(2):
                            for l in range(2):
                                h = 2 * p + l
                                nc.sync.dma_start(
                                    out=qaug[p][64 * l + R:64 * l + R + 1,
                                                128 * i:128 * i + 128],
                                    in_=nmt_sb[h:h + 1, :])

                def emit_stav(c):
                    for p in range(2):
                        avp = [psav.tile([DH + 1, 512], F32, tag=f"av{l}",
                                         name=f"av{l}") for l in range(2)]
                        njt = 4 * c + 4
                        for j in range(njt):
                            for l in range(2):
                                h = 2 * p + l
                                stp = psT.tile([128, 512], F32,
                                               tag=f"sT{l}", name=f"sT{l}",
                                               bufs=(2 if l == 0 else 1))
                                nc.tensor.matmul(
                                    stp[:],
                                    kaug[p][64 * l:64 * l + R + 17,
                                            128 * j:128 * j + 128],
                                    qaug[p][64 * l:64 * l + R + 17,
                                            512 * c:512 * c + 512],
                                    start=True, stop=True,
                                    tile_position=(64 * l, 0))
                                if j // 4 == c:
                                    a = 128 * (j - 4 * c)
                                    nc.vector.tensor_add(
                                        stp[:, a:a + 128],
                                        stp[:, a:a + 128], trik_t[:])
                                pt = pst.tile([128, 512], F32R, tag="pt",
                                              name="pt")
                                nc.scalar.activation(pt[:], stp[:], EXP)
                                nc.tensor.matmul(
                                    avp[l][:], vaug[h][j][:], pt[:],
                                    start=(j == 0), stop=(j == njt - 1))
                        rcp = pcx.tile([2, 512], F32R, tag="rcp", name="rcp")
                        ctxf = pcx.tile([128, 512], F32, tag="ctxf",
                                        name="ctxf")
                        for l in range(2):
                            rc1 = pcx.tile([1, 512], F32R, tag=f"rc{l}",
                                           name=f"rc{l}")
                            nc.vector.reciprocal(rc1[:],
                                                 avp[l][DH:DH + 1, :])
                            nc.sync.dma_start(out=rcp[l:l + 1, :],
                                              in_=rc1[:])
                            nc.scalar.copy(ctxf[64 * l:64 * l + 64, :],
                                           avp[l][0:DH, :])
                        scl = psw.tile([128, 512], F32, tag="st0",
                                       name="scl")
                        nc.tensor.matmul(scl[:], sel2_t[:], rcp[:],
                                         start=True, stop=True)
                        nc.vector.tensor_mul(ctxr[p][c][:], ctxf[:], scl[:])

                def emit_oproj(c):
                    for ot in range(8):
                        yps = psw.tile([128, 512], F32, tag="st1",
                                       name="yps")
                        for p in range(2):
                            nc.tensor.matmul(
                                yps[:],
                                wo_t[p][:, 128 * ot:128 * ot + 128],
                                ctxr[p][c][:],
                                start=(p == 0), stop=(p == 1))
                        ysb = pcx.tile([128, 512], F32, tag="ysb",
                                       name="ysb")
                        nc.scalar.copy(ysb[:], yps[:])
                        nc.sync.dma_start(
                            out=yT[128 * ot:128 * ot + 128,
                                   512 * c:512 * c + 512],
                            in_=ysb[:])

                emit_stats(0)
                for c in range(NCH):
                    if c + 1 < NCH:
                        emit_stats(c + 1)
                    emit_stav(c)
                    emit_oproj(c)

    nc.compile()
    return nc


def _consts():
    indq = np.zeros((NT, T), np.float32)
    for j in range(NT):
        indq[j, :128 * j] = NEG
    okq = np.zeros((NT + 1, T), np.float32)
    okq[0] = 1.0
    for j in range(NT):
        okq[1 + j, 128 * j:128 * j + 128] = 1.0
    triq = np.triu(np.full((128, 128), NEG, np.float32), 1)
    trik = np.tril(np.full((128, 128), NEG, np.float32), -1)
    sel2 = np.zeros((2, 128), np.float32)
    sel2[0, :64] = 1.0
    sel2[1, 64:] = 1.0
    ibf = np.eye(128).astype(ml_dtypes.bfloat16)
    return indq, okq, triq, trik, sel2, ibf


def kernel(x, Wq, bq, Wk, bk, Wv, bv, Wo, bo, Wq_lsr, Wk_lsr):
    from concourse.bass_utils import run_bass_kernel_spmd

    if "nc" not in _cache:
        _cache["nc"] = _build()
    nc = _cache["nc"]

    x = np.asarray(x, np.float32)
    Wq = np.asarray(Wq, np.float32)
    Wk = np.asarray(Wk, np.float32)
    Wv = np.asarray(Wv, np.float32)
    Wo = np.asarray(Wo, np.float32)
    bq = np.asarray(bq, np.float32)
    bk = np.asarray(bk, np.float32)
    bv = np.asarray(bv, np.float32)
    bo = np.asarray(bo, np.float32)
    Wq_lsr = np.asarray(Wq_lsr, np.float32)
    Wk_lsr = np.asarray(Wk_lsr, np.float32)

    indq, okq, triq, trik, sel2, ibf = _consts()
    in_maps = []
    for core in range(NCORES):
        b, g = divmod(core, 4)
        hs = HPC * g
        cols = slice(DH * hs, DH * hs + OC)
        # per-head lsr weights side by side: [DH, HPC*R]
        wql = np.ascontiguousarray(
            Wq_lsr[hs:hs + HPC].transpose(1, 0, 2).reshape(DH, HPC * R))
        wkl = np.ascontiguousarray(
            Wk_lsr[hs:hs + HPC].transpose(1, 0, 2).reshape(DH, HPC * R))
        in_maps.append({
            "xT": np.ascontiguousarray(x[b].T),
            "wq": np.ascontiguousarray(Wq[:, cols]),
            "wk": np.ascontiguousarray(Wk[:, cols]),
            "wv": np.ascontiguousarray(Wv[:, cols]),
            "wo": np.ascontiguousarray(Wo[cols, :]),
            "wql": wql, "wkl": wkl,
            "bq": np.ascontiguousarray(bq[cols, None]),
            "bk": np.ascontiguousarray(bk[cols, None]),
            "indq": indq, "okq": okq, "triq": triq, "trik": trik,
            "sel2": sel2, "ibf": ibf,
        })

    res = run_bass_kernel_spmd(nc, in_maps, list(range(NCORES)),
                               **_cache.get("run_kwargs", {}))
    _cache["last_results"] = res

    y = np.zeros((B, T, D), np.float32)
    for core in range(NCORES):
        b = core // 4
        y[b] += res.results[core]["yT"].T
    y += (bv @ Wo + bo)[None, None, :]
    return y


# revision 12
# speedup vs baseline: 1.0305x; 1.0305x over previous
"""Multi-head low-rank-score (LSR) causal attention on 8 trn2 NeuronCores.

Sharding: core = 4*b + g owns batch b and heads [4g, 4g+4).
Each core: q/k/v projections for its 256 head-dims, low-rank score
projections, causal softmax attention, and a partial o_proj
(its 256 ctx dims x full 1024 out dims). Host sums the 4 partials
per batch and adds biases.

All matmuls run in float32r (full-rate fp32, ~13-bit mantissa).
Softmax: two score passes -
  pass 1 (q-layout) computes the per-query causal row max on DVE;
  pass 2 (k-layout) computes S^T with the max subtraction and the
  block-level causal mask folded into the matmul via augmented
  contraction rows, then exp() on ScalarE writes P^T straight to SBUF.
AV multiplies V (augmented with a ones column -> softmax denominators
come out of the same matmul) by P^T, and the division is applied via a
reciprocal + rank-2 broadcast matmul + one DVE multiply.
"""

import numpy as np
import ml_dtypes

B = 2
T = 2048
D = 1024
H = 16
DH = 64
R = 32
HPC = 4  # heads per core
OC = HPC * DH  # 256 out-cols per core
NCORES = 8
SCALE = 1.0 / float(np.sqrt(np.float32(R)))
NEG = -30000.0
NT = T // 128  # 16 key/query tiles
NCH = T // 512  # 4 query chunks

_cache = {}


def _build():
    import concourse.bacc as bacc
    import concourse.mybir as mybir
    from concourse.tile import TileContext

    F32 = mybir.dt.float32
    F32R = mybir.dt.float32r
    BF16 = mybir.dt.bfloat16
    EXP = mybir.ActivationFunctionType.Exp
    COPY = mybir.ActivationFunctionType.Copy
    IDENT = mybir.ActivationFunctionType.Identity
    MAX = mybir.AluOpType.max
    AXX = mybir.AxisListType.X

    nc = bacc.Bacc("TRN2", target_bir_lowering=False, debug=False,
                   num_devices=NCORES)

    xT = nc.declare_dram_parameter("xT", [D, T], F32R, isOutput=False)
    wq = nc.declare_dram_parameter("wq", [D, OC], F32R, isOutput=False)
    wk = nc.declare_dram_parameter("wk", [D, OC], F32R, isOutput=False)
    wv = nc.declare_dram_parameter("wv", [D, OC], F32R, isOutput=False)
    wo = nc.declare_dram_parameter("wo", [OC, D], BF16, isOutput=False)
    wql = nc.declare_dram_parameter("wql", [DH, HPC * R], F32R, isOutput=False)
    wkl = nc.declare_dram_parameter("wkl", [DH, HPC * R], F32R, isOutput=False)
    bq = nc.declare_dram_parameter("bq", [OC, 1], F32, isOutput=False)
    bk = nc.declare_dram_parameter("bk", [OC, 1], F32, isOutput=False)
    # [16, T] row j': NEG where t < 128*j' else 0
    indq = nc.declare_dram_parameter("indq", [NT, T], F32R, isOutput=False)
    # [17, T]: row 0 = ones; rows 1+j': 1.0 on k-tile j' cols else 0
    okq = nc.declare_dram_parameter("okq", [NT + 1, T], F32R, isOutput=False)
    triq = nc.declare_dram_parameter("triq", [128, 128], F32, isOutput=False)
    trik = nc.declare_dram_parameter("trik", [128, 128], F32, isOutput=False)
    sel2 = nc.declare_dram_parameter("sel2", [2, 128], F32R, isOutput=False)
    ibf = nc.declare_dram_parameter("ibf", [128, 128], BF16, isOutput=False)
    yT = nc.declare_dram_parameter("yT", [D, T], F32, isOutput=True)

    with TileContext(nc) as tc:
        with (
            nc.allow_low_precision(reason="f32r reciprocal / bf16 row-max"),
            tc.tile_pool(name="persist", bufs=1) as pp,
        ):
            # ---- persistent SBUF tiles
            wq_t = [pp.tile([128, OC], F32R, tag=f"wq{i}", name=f"wq{i}") for i in range(8)]
            wk_t = [pp.tile([128, OC], F32R, tag=f"wk{i}", name=f"wk{i}") for i in range(8)]
            wv_t = [pp.tile([128, OC], F32R, tag=f"wv{i}", name=f"wv{i}") for i in range(8)]
            # lsr weights duplicated at partition bases 0 and 64 (row packing)
            wql_t = pp.tile([128, HPC * R], F32R, tag="wql")
            wkl_t = pp.tile([128, HPC * R], F32R, tag="wkl")
            bq_t = [pp.tile([128, 1], F32, tag=f"bq{i}", name=f"bq{i}") for i in range(2)]
            bk_t = [pp.tile([128, 1], F32, tag=f"bk{i}", name=f"bk{i}") for i in range(2)]
            triq_t = pp.tile([128, 128], F32, tag="triq")
            trik_t = pp.tile([128, 128], F32, tag="trik")
            sel2_t = pp.tile([2, 128], F32R, tag="sel2")
            ibf_t = pp.tile([128, 128], BF16, tag="ibf")
            # augmented lr tiles, one per head pair p (heads 2p, 2p+1)
            # rows [64l, 64l+32): scale*q_lrT / k_lrT of head 2p+l
            # row 64l+32: -m (q side) / ones (k side)
            # rows [64l+33, 64l+49): indq (q side) / selk (k side)
            qaug = [pp.tile([128, T], F32R, tag=f"qaug{p}", name=f"qaug{p}") for p in range(2)]
            kaug = [pp.tile([128, T], F32R, tag=f"kaug{p}", name=f"kaug{p}") for p in range(2)]
            # V augmented with ones column, per head x key tile
            vaug = [[pp.tile([128, DH + 1], BF16, tag=f"va{h}_{j}", name=f"va{h}_{j}")
                     for j in range(NT)] for h in range(HPC)]
            # scaled ctx^T ready for o_proj: [pair][chunk]
            ctxr = [[pp.tile([128, 512], BF16, tag=f"cx{p}_{c}", name=f"cx{p}_{c}")
                     for c in range(NCH)] for p in range(2)]
            wo_t = [pp.tile([128, D], BF16, tag=f"wo{p}", name=f"wo{p}") for p in range(2)]

            for i in range(8):
                nc.sync.dma_start(out=wq_t[i][:], in_=wq[128 * i:128 * i + 128, :])
                nc.sync.dma_start(out=wk_t[i][:], in_=wk[128 * i:128 * i + 128, :])
                nc.sync.dma_start(out=wv_t[i][:], in_=wv[128 * i:128 * i + 128, :])
            for l in range(2):
                nc.sync.dma_start(out=wql_t[64 * l:64 * l + DH, :], in_=wql[:])
                nc.sync.dma_start(out=wkl_t[64 * l:64 * l + DH, :], in_=wkl[:])
            for i in range(2):
                nc.sync.dma_start(out=bq_t[i][:], in_=bq[128 * i:128 * i + 128, :])
                nc.sync.dma_start(out=bk_t[i][:], in_=bk[128 * i:128 * i + 128, :])
            nc.sync.dma_start(out=triq_t[:], in_=triq[:])
            nc.sync.dma_start(out=trik_t[:], in_=trik[:])
            nc.sync.dma_start(out=sel2_t[:], in_=sel2[:])
            nc.sync.dma_start(out=ibf_t[:], in_=ibf[:])
            for p in range(2):
                nc.sync.dma_start(out=wo_t[p][:], in_=wo[128 * p:128 * p + 128, :])
                for l in range(2):
                    nc.sync.dma_start(out=qaug[p][64 * l + 33:64 * l + 49, :],
                                      in_=indq[:])
                    nc.sync.dma_start(out=kaug[p][64 * l + 32:64 * l + 49, :],
                                      in_=okq[:])

            # ---- phase 1: projections (uses xT; QT/KT transient)
            with (
                tc.tile_pool(name="px", bufs=1) as px,
                tc.tile_pool(name="pqk", bufs=2) as pqk,
                tc.tile_pool(name="ps1", bufs=2, space="PSUM") as ps1,
                tc.tile_pool(name="psl", bufs=2, space="PSUM") as psl,
            ):
                xt_t = [px.tile([128, T], F32R, tag=f"x{i}", name=f"x{i}") for i in range(8)]
                for i in range(8):
                    nc.sync.dma_start(out=xt_t[i][:],
                                      in_=xT[128 * i:128 * i + 128, :])

                # V: [t-tile, 256] accumulating 8 k-tiles
                for tt in range(NT):
                    vps = ps1.tile([128, OC], F32, tag="vps")
                    for kk in range(8):
                        nc.tensor.matmul(
                            vps[:], xt_t[kk][:, 128 * tt:128 * tt + 128],
                            wv_t[kk][:], start=(kk == 0), stop=(kk == 7))
                    for h in range(HPC):
                        nc.scalar.copy(vaug[h][tt][:, 0:DH],
                                       vps[:, DH * h:DH * h + DH])
                        nc.vector.memset(vaug[h][tt][:, DH:DH + 1], 1.0)

                # QT/KT oc-tiles -> lsr -> aug tiles; QT/KT slots recycled
                for side in range(2):  # 0 = q, 1 = k
                    w_t = wq_t if side == 0 else wk_t
                    b_t = bq_t if side == 0 else bk_t
                    lsr_w = wql_t if side == 0 else wkl_t
                    aug = qaug if side == 0 else kaug
                    evac_scale = SCALE if side == 0 else 1.0
                    for ot in range(2):  # oc tile = head pair p = ot
                        qk_sb = pqk.tile([128, T], F32R, tag="qkt")
                        for ch in range(NCH):
                            pps = ps1.tile([128, 512], F32, tag="pps")
                            for kk in range(8):
                                nc.tensor.matmul(
                                    pps[:],
                                    w_t[kk][:, 128 * ot:128 * ot + 128],
                                    xt_t[kk][:, 512 * ch:512 * ch + 512],
                                    start=(kk == 0), stop=(kk == 7))
                            nc.scalar.activation(
                                qk_sb[:, 512 * ch:512 * ch + 512], pps[:],
                                IDENT, bias=b_t[ot][:], scale=1.0)
                        # lsr for the two heads in this oc tile (row-packed
                        # at partition bases 0 / 64)
                        for ch in range(NCH):
                            for l in range(2):
                                h = 2 * ot + l
                                lps = psl.tile([R, 512], F32, tag=f"lps{l}", name=f"lps{l}")
                                nc.tensor.matmul(
                                    lps[:],
                                    lsr_w[64 * l:64 * l + DH,
                                          R * h:R * h + R],
                                    qk_sb[64 * l:64 * l + DH,
                                          512 * ch:512 * ch + 512],
                                    start=True, stop=True,
                                    tile_position=(64 * l, 0))
                                nc.scalar.activation(
                                    aug[ot][64 * l:64 * l + R,
                                            512 * ch:512 * ch + 512],
                                    lps[:], COPY, scale=evac_scale)

            # ---- phases 2-4 merged: stats / S^T+AV / o_proj interleaved
            # per 512-query chunk so the PE never idles long enough to
            # re-throttle. PSUM: sT0(2) + sT1(1) + st0/st1(2) + av0/av1(2)
            # + nmt(1) = 8 banks; scl shares st0, yps shares st1.
            with (
                tc.tile_pool(name="psw", bufs=1, space="PSUM") as psw,
                tc.tile_pool(name="psT", bufs=2, space="PSUM") as psT,
                tc.tile_pool(name="psav", bufs=1, space="PSUM") as psav,
                tc.tile_pool(name="psn", bufs=1, space="PSUM") as psn,
                tc.tile_pool(name="pmx", bufs=2) as pmx,
                tc.tile_pool(name="pst", bufs=6) as pst,
                tc.tile_pool(name="pcx", bufs=2) as pcx,
            ):
                def emit_stats(c):
                    for ii in range(4):
                        i = 4 * c + ii
                        nchunks = i // 4 + 1
                        mx = pmx.tile([128, HPC * 4], F32, tag="mx",
                                      name="mx")
                        negm = pmx.tile([128, HPC], BF16, tag="negm",
                                        name="negm")
                        for p in range(2):
                            for l in range(2):
                                h = 2 * p + l
                                for cc in range(nchunks):
                                    ncols = min(512, 128 * (i + 1) - 512 * cc)
                                    sps = psw.tile([128, 512], F32,
                                                   tag=f"st{l}",
                                                   name=f"st{l}")
                                    nc.tensor.matmul(
                                        sps[:, 0:ncols],
                                        qaug[p][64 * l:64 * l + R,
                                                128 * i:128 * i + 128],
                                        kaug[p][64 * l:64 * l + R,
                                                512 * cc:512 * cc + ncols],
                                        start=True, stop=True,
                                        tile_position=(64 * l, 0))
                                    if cc == nchunks - 1:
                                        a = ncols - 128
                                        nc.vector.tensor_add(
                                            sps[:, a:a + 128],
                                            sps[:, a:a + 128], triq_t[:])
                                    nc.vector.tensor_reduce(
                                        mx[:, 4 * h + cc:4 * h + cc + 1],
                                        sps[:, 0:ncols], axis=AXX, op=MAX)
                        for h in range(HPC):
                            nc.vector.tensor_reduce(
                                negm[:, h:h + 1],
                                mx[:, 4 * h:4 * h + nchunks],
                                axis=AXX, op=MAX, negate=True)
                        nmt = psn.tile([HPC, 128], BF16, tag="nmt",
                                       name="nmt")
                        nc.tensor.transpose(nmt[:], negm[:], ibf_t[:])
                        nmt_sb = pmx.tile([HPC, 128], F32R, tag="nmt_sb",
                                          name="nmt_sb")
                        nc.scalar.copy(nmt_sb[:], nmt[:])
                        for p in range(2):
                            for l in range(2):
                                h = 2 * p + l
                                nc.sync.dma_start(
                                    out=qaug[p][64 * l + R:64 * l + R + 1,
                                                128 * i:128 * i + 128],
                                    in_=nmt_sb[h:h + 1, :])

                def emit_stav(c):
                    for p in range(2):
                        avp = [psav.tile([DH + 1, 512], F32, tag=f"av{l}",
                                         name=f"av{l}") for l in range(2)]
                        njt = 4 * c + 4
                        for j in range(njt):
                            for l in range(2):
                                h = 2 * p + l
                                stp = psT.tile([128, 512], F32,
                                               tag=f"sT{l}", name=f"sT{l}",
                                               bufs=(2 if l == 0 else 1))
                                nc.tensor.matmul(
                                    stp[:],
                                    kaug[p][64 * l:64 * l + R + 17,
                                            128 * j:128 * j + 128],
                                    qaug[p][64 * l:64 * l + R + 17,
                                            512 * c:512 * c + 512],
                                    start=True, stop=True,
                                    tile_position=(64 * l, 0))
                                if j // 4 == c:
                                    a = 128 * (j - 4 * c)
                                    nc.vector.tensor_add(
                                        stp[:, a:a + 128],
                                        stp[:, a:a + 128], trik_t[:])
                                pt = pst.tile([128, 512], BF16, tag="pt",
                                              name="pt")
                                nc.scalar.activation(pt[:], stp[:], EXP)
                                nc.tensor.matmul(
                                    avp[l][:], vaug[h][j][:], pt[:],
                                    start=(j == 0), stop=(j == njt - 1))
                        rcp = pcx.tile([2, 512], F32R, tag="rcp", name="rcp")
                        ctxf = pcx.tile([128, 512], F32, tag="ctxf",
                                        name="ctxf")
                        for l in range(2):
                            rc1 = pcx.tile([1, 512], F32R, tag=f"rc{l}",
                                           name=f"rc{l}")
                            nc.vector.reciprocal(rc1[:],
                                                 avp[l][DH:DH + 1, :])
                            nc.sync.dma_start(out=rcp[l:l + 1, :],
                                              in_=rc1[:])
                            nc.scalar.copy(ctxf[64 * l:64 * l + 64, :],
                                           avp[l][0:DH, :])
                        scl = psw.tile([128, 512], F32, tag="st0",
                                       name="scl")
                        nc.tensor.matmul(scl[:], sel2_t[:], rcp[:],
                                         start=True, stop=True)
                        nc.vector.tensor_mul(ctxr[p][c][:], ctxf[:], scl[:])

                def emit_oproj(c):
                    for ot in range(8):
                        yps = psw.tile([128, 512], F32, tag="st1",
                                       name="yps")
                        for p in range(2):
                            nc.tensor.matmul(
                                yps[:],
                                wo_t[p][:, 128 * ot:128 * ot + 128],
                                ctxr[p][c][:],
                                start=(p == 0), stop=(p == 1))
                        ysb = pcx.tile([128, 512], F32, tag="ysb",
                                       name="ysb")
                        nc.scalar.copy(ysb[:], yps[:])
                        nc.sync.dma_start(
                            out=yT[128 * ot:128 * ot + 128,
                                   512 * c:512 * c + 512],
                            in_=ysb[:])

                emit_stats(0)
                for c in range(NCH):
                    if c + 1 < NCH:
                        emit_stats(c + 1)
                    emit_stav(c)
                    emit_oproj(c)

    nc.compile()
    return nc


def _consts():
    indq = np.zeros((NT, T), np.float32)
    for j in range(NT):
        indq[j, :128 * j] = NEG
    okq = np.zeros((NT + 1, T), np.float32)
    okq[0] = 1.0
    for j in range(NT):
        okq[1 + j, 128 * j:128 * j + 128] = 1.0
    triq = np.triu(np.full((128, 128), NEG, np.float32), 1)
    trik = np.tril(np.full((128, 128), NEG, np.float32), -1)
    sel2 = np.zeros((2, 128), np.float32)
    sel2[0, :64] = 1.0
    sel2[1, 64:] = 1.0
    ibf = np.eye(128).astype(ml_dtypes.bfloat16)
    return indq, okq, triq, trik, sel2, ibf


def kernel(x, Wq, bq, Wk, bk, Wv, bv, Wo, bo, Wq_lsr, Wk_lsr):
    from concourse.bass_utils import run_bass_kernel_spmd

    if "nc" not in _cache:
        _cache["nc"] = _build()
    nc = _cache["nc"]

    x = np.asarray(x, np.float32)
    Wq = np.asarray(Wq, np.float32)
    Wk = np.asarray(Wk, np.float32)
    Wv = np.asarray(Wv, np.float32)
    Wo = np.asarray(Wo, np.float32)
    bq = np.asarray(bq, np.float32)
    bk = np.asarray(bk, np.float32)
    bv = np.asarray(bv, np.float32)
    bo = np.asarray(bo, np.float32)
    Wq_lsr = np.asarray(Wq_lsr, np.float32)
    Wk_lsr = np.asarray(Wk_lsr, np.float32)

    indq, okq, triq, trik, sel2, ibf = _consts()
    in_maps = []
    for core in range(NCORES):
        b, g = divmod(core, 4)
        hs = HPC * g
        cols = slice(DH * hs, DH * hs + OC)
        # per-head lsr weights side by side: [DH, HPC*R]
        wql = np.ascontiguousarray(
            Wq_lsr[hs:hs + HPC].transpose(1, 0, 2).reshape(DH, HPC * R))
        wkl = np.ascontiguousarray(
            Wk_lsr[hs:hs + HPC].transpose(1, 0, 2).reshape(DH, HPC * R))
        in_maps.append({
            "xT": np.ascontiguousarray(x[b].T),
            "wq": np.ascontiguousarray(Wq[:, cols]),
            "wk": np.ascontiguousarray(Wk[:, cols]),
            "wv": np.ascontiguousarray(Wv[:, cols]),
            "wo": np.ascontiguousarray(Wo[cols, :]).astype(ml_dtypes.bfloat16),
            "wql": wql, "wkl": wkl,
            "bq": np.ascontiguousarray(bq[cols, None]),
            "bk": np.ascontiguousarray(bk[cols, None]),
            "indq": indq, "okq": okq, "triq": triq, "trik": trik,
            "sel2": sel2, "ibf": ibf,
        })

    res = run_bass_kernel_spmd(nc, in_maps, list(range(NCORES)),
                               **_cache.get("run_kwargs", {}))
    _cache["last_results"] = res

    y = np.zeros((B, T, D), np.float32)
    for core in range(NCORES):
        b = core // 4
        y[b] += res.results[core]["yT"].T
    y += (bv @ Wo + bo)[None, None, :]
    return y
